# revision 30
# baseline (speedup 1.0000x reference)
"""Trainium2 Bass kernel for ProbSparse multi-head attention (L_Q = 1).

Math: with L_Q=1 the reference's top-k/sampling machinery is identity
(top-1 of a length-1 axis is index 0 and the scatter overwrites the whole
context), so the computation reduces to single-query attention:

  out[b] = concat_h( softmax((q Wq)_h . (k Wk)_h^T / 8) @ (v Wv)_h ) @ Wo + bo

Key algebraic restructuring (L_Q = 1 => low rank):
  scores[b,h,s] = k[b,s,:] . r[b,h,:]      with r[b,h] = Wk_h @ (qh[b,h]/8)
  w[b,h,:]      = sum_s attn[b,h,s] v[b,s,:]
  out[b]        = rowsum_h(masked((w/Z) Wv)) @ Wo + bo

so the big k/v tensors are consumed by exactly one streaming pass each and
never projected through the weight matrices (64x fewer FLOPs).

v2: k arrives pre-transposed (kT, hidden-major) and pre-cast to bf16 from
the host, v pre-cast to bf16, weights pre-cast to bf16.  This removes all
device-side f32->bf16 casts and all PE transposes of k (the v1 bottleneck:
~500us of PE occupancy and 84MB of DMA drop to ~190us PE / 38MB DMA).
The tail (w -> u -> out) is batched over both local batches so Wv / Wo
stream through the PE once per core instead of once per batch.

Sharding: data-parallel over batch, 2 batches per core, 8 cores.

Sync-wait limits (some instruction encodings accept a single semaphore
wait) are handled as in v1: PE transposes are preceded by an
absorber ldweights+matmul pair that soaks up cross-engine waits, and DVE
touch-copies of DMA'd constants early in the program teach the DVE clock
about those DMA ticks before scalar_tensor_tensor needs them.
"""

import sys

sys.path.insert(0, "/opt/trn_rl_repo")

import numpy as np
import ml_dtypes

import concourse.bass as bass
import concourse.mybir as mybir
from bass_rust import add_dep_helper
import concourse.tile_sem_assignment as _tsa
from concourse.tile import TileContext
from concourse import bass_utils


# ---- framework patch: the kernel-tail drain aggregates one semaphore wait
# per active proc, exceeding the 1-wait DRAIN encoding. Split the waits
# across a chain of single-wait drains.
from concourse.tile import TileContext as _TC
from concourse.vector_clock import ScopedClock as _SC

def _split_drain_and_barrier(self, tick_clock, wait_clock):
    drain_inst = self.nc.sync.drain()
    wait_clock.add_sem_waits(drain_inst.ins, _SC({None: tick_clock.global_clock}))
    si = drain_inst.ins.sync_info
    if si is not None and si.on_wait and len(si.on_wait) > 1:
        waits = list(si.on_wait)
        si.on_wait = waits[:1]
        for w in waits[1:]:
            d2 = self.nc.sync.drain()
            s2 = d2.ins.sync_info
            if s2 is None:
                d2.ins.sync_info = type(si)(on_wait=[w], on_update=[])
            else:
                s2.on_wait = [w]
    self.nc.all_engine_barrier()
    assert self.sems is not None
    popped = self.nc._tile_sem_poison_stack.pop()
    assert popped is self._sem_poison
    self.nc.clear_and_free_semaphores(list(self.sems.allocated().values()))
    self.nc.all_engine_barrier()

_TC._drain_and_barrier = _split_drain_and_barrier

B, H, DH, HID, LK = 16, 16, 64, 1024, 4096
NCORES = 8
BL = B // NCORES            # batches per core
NCH = HID // 128            # 8 hidden chunks
NT = LK // 1024             # 4 stream tiles of 1024 seq positions
H2 = BL * H                 # packed (b,h) rows for the r projection
HS = 64                     # tail stacking: batch b at partitions b*32

f32 = mybir.dt.float32
bf16 = mybir.dt.bfloat16
FT = mybir.ActivationFunctionType
AX = mybir.AxisListType

bf16_np = ml_dtypes.bfloat16


def build_nc():
    # one DMA-completion semaphore lane per DGE type: consumers then never
    # accumulate multi-lane DMA waits (several instruction structs allow
    # only 1-2 sync waits).
    _tsa.NUM_HWDGE_SEMS = 1
    _tsa.NUM_SWDGE_GLOBAL_SEMS = 1

    nc = bass.Bass("TRN2")

    kT_d = nc.dram_tensor("kT_loc", [BL, HID, LK], bf16, kind="ExternalInput")
    v_d = nc.dram_tensor("v_loc", [BL, LK, HID], bf16, kind="ExternalInput")
    Wq_d = nc.dram_tensor("Wq", [HID, HID], bf16, kind="ExternalInput")
    WkT_d = nc.dram_tensor("WkT", [HID, HID], bf16, kind="ExternalInput")
    Wv_d = nc.dram_tensor("Wv", [HID, HID], bf16, kind="ExternalInput")
    Wo_d = nc.dram_tensor("Wo", [HID, HID], bf16, kind="ExternalInput")
    blob_d = nc.dram_tensor("blob", [128, 3230], bf16, kind="ExternalInput")
    out_d = nc.dram_tensor("out_loc", [BL, HID], f32, kind="ExternalOutput")

    with TileContext(nc) as tc:
        with tc.tile_pool(name="main", bufs=1) as mp, \
             tc.tile_pool(name="ps", bufs=1, space="PSUM") as pp:

            # ---- constants: one packed DMA (the FIFO chain makes many
            # small const DMAs cost ~1.5us each in serialized latency) ----
            blob = mp.tile([128, 3230], bf16, tag="blob")
            nc.scalar.dma_start(out=blob, in_=blob_d[:, :])
            idb = blob[:, 0:128]
            mask2 = blob[0:HS, 128:128 + HID]
            boB = blob[0:1, 1152:1152 + HID]
            one14 = blob[0:1, 3202:3206]
            bvT = blob[:, 3206:3206 + NCH]

            # DVE touch-copies: teach the DVE clock the const-DMA ticks so
            # later 1-wait DVE structs (scalar_tensor_tensor) don't need a
            # DMA wait slot of their own.
            scratch = mp.tile([128, 8], f32, tag="scratch")
            nc.vector.tensor_copy(scratch[0:HS, 0:1], mask2[:, 0:1])
            nc.vector.tensor_copy(scratch[:, 1:2], bvT[:, 0:1])
            scratch2 = mp.tile([1, 8], f32, tag="scratch2")
            rT_sb = mp.tile([128, NCH, H2], bf16, tag="rT")

            # dedicated never-read psum tile: every absorber dummy writes
            # here, so each dummy carries only the PE WAW-drain wait of its
            # predecessor (a chain) and no DVE WAR.
            dmy_ps = pp.tile([1, 64], f32, tag="dmy")

            # ---- setup: qh = Wq^T qT (+bq), r = WkT^T Qt ----
            # setup-only SBUF lives in a nested pool so its 37KB/partition is
            # reused by the stream tiles afterwards.
            sp_ctx = tc.tile_pool(name="setup", bufs=1)
            sp = sp_ctx.__enter__()
            qT_sb = blob[:, 3214:3214 + NCH * BL].rearrange(
                "p (ch b) -> p ch b", b=BL)
            Wq_sb = sp.tile([128, NCH, HID], bf16, tag="Wq")
            nc.scalar.dma_start(
                out=Wq_sb, in_=Wq_d[:, :].rearrange("(ch p) h -> p ch h", p=128))
            WkT_sb = sp.tile([128, NCH, HID], bf16, tag="WkT")
            nc.scalar.dma_start(
                out=WkT_sb, in_=WkT_d[:, :].rearrange("(ch p) h -> p ch h", p=128))
            bqB = blob[0:1, 2176:2176 + HID]
            one12 = blob[0:1, 3200:3200 + BL]
            # qh[b, :]: lhsT = qT chunk [128, BL], moving = Wq chunk.
            psum_qh = pp.tile([32, HID], f32, tag="w")
            for ch in range(NCH):
                for hf in range(2):
                    nc.tensor.matmul(
                        psum_qh[0:BL, hf * 512:(hf + 1) * 512],
                        qT_sb[:, ch, :],
                        Wq_sb[:, ch, hf * 512:(hf + 1) * 512],
                        start=(ch == 0), stop=False)
            for hf in range(2):
                nc.tensor.matmul(
                    psum_qh[0:BL, hf * 512:(hf + 1) * 512],
                    one12, bqB[0:1, hf * 512:(hf + 1) * 512],
                    start=False, stop=(hf == 1))
            qh_sb = sp.tile([BL, HID], bf16, tag="qh")
            nc.vector.tensor_copy(qh_sb, psum_qh[0:BL, :])

            # transpose qh -> qhT [hd, b] (PE, absorber pattern)
            ldw_q = nc.tensor.ldweights(qh_sb[0:BL, 0:1])
            dmy_q = nc.tensor.matmul(dmy_ps[0:1, 0:BL], qh_sb[0:BL, 0:1],
                                     idb[0:BL, 0:BL], start=True, stop=True)
            add_dep_helper(dmy_q.ins, ldw_q.ins, reason="absorb-chain")
            psum_qt = pp.tile([128, NCH, BL], bf16, tag="tp")
            for ch in range(NCH):
                tp_i = nc.tensor.transpose(
                    psum_qt[:, ch, :],
                    qh_sb[0:BL, ch * 128:(ch + 1) * 128], idb[0:BL, 0:BL])
                add_dep_helper(tp_i.ins, dmy_q.ins, reason="absorb")
            qhT_sb = sp.tile([128, NCH, BL], bf16, tag="qhT")
            nc.vector.tensor_copy(qhT_sb, psum_qt)

            # Qt: block-diag expansion [hd, (ch, b, h)], h == head(hd)
            Qt_sb = sp.tile([128, NCH, BL, H], bf16, tag="Qt")
            nc.vector.memset(Qt_sb, 0.0)
            for m in range(NCH):
                for g in range(2):
                    h = 2 * m + g
                    nc.vector.tensor_copy(
                        Qt_sb[g * 64:(g + 1) * 64, m, :, h],
                        qhT_sb[g * 64:(g + 1) * 64, m, :])

            # rTT[(b h), c] = sum_hd Qt[hd, (b h)] WkT[hd, c]
            psum_rTT = pp.tile([32, HID], f32, tag="w")
            ldw_wk = nc.tensor.ldweights(WkT_sb[:, 0, 0:1])
            first_rtt = [True]
            for ch in range(NCH):
                for hf in range(2):
                    mm = nc.tensor.matmul(
                        psum_rTT[0:H2, hf * 512:(hf + 1) * 512],
                        Qt_sb[:, ch, :, :],
                        WkT_sb[:, ch, hf * 512:(hf + 1) * 512],
                        start=(ch == 0), stop=(ch == NCH - 1))
                    if first_rtt[0]:
                        add_dep_helper(mm.ins, ldw_wk.ins, reason="absorb")
                        first_rtt[0] = False
            rTT_sb = sp.tile([H2, HID], bf16, tag="rTT")
            nc.vector.tensor_copy(rTT_sb, psum_rTT[0:H2, :])

            # transpose rTT -> rT [c, (b h)]
            ldw_r = nc.tensor.ldweights(rTT_sb[0:H2, 0:1])
            dmy_r = nc.tensor.matmul(dmy_ps[0:1, 0:H2], rTT_sb[0:H2, 0:1],
                                     idb[0:H2, 0:H2], start=True, stop=True)
            add_dep_helper(dmy_r.ins, ldw_r.ins, reason="absorb-chain")
            psum_rt = pp.tile([128, NCH, H2], bf16, tag="tp")
            for cj in range(NCH):
                tp_i = nc.tensor.transpose(
                    psum_rt[:, cj, :],
                    rTT_sb[0:H2, cj * 128:(cj + 1) * 128], idb[0:H2, 0:H2])
                add_dep_helper(tp_i.ins, dmy_r.ins, reason="absorb")
            nc.vector.tensor_copy(rT_sb, psum_rt)
            sp_ctx.__exit__(None, None, None)
            # PE marker into the dmy_ps corner, then an ACT touch of that
            # corner: the touch carries one PE RAW wait and ratchets the ACT
            # clock over all setup PE work.
            nc.tensor.matmul(dmy_ps[0:1, 48:49], rTT_sb[0:1, 0:1],
                             rTT_sb[0:1, 0:1], start=True, stop=True)
            nc.scalar.copy(scratch2[0:1, 0:1], dmy_ps[0:1, 48:49])

            # ---- streaming batches ----
            psum_w = [None, None]
            Zi2 = mp.tile([HS, 1], f32, tag="Zi2")
            nc.vector.memset(Zi2, 0.0)
            last_psum_at = [None]
            attnT_sb_prev = [None]
            for bl in range(BL):
                if bl > 0:
                    # ratchet ACT past batch bl-1's V phase (the marker) so
                    # this batch's kt/vt triggers carry only their DMA WAW.
                    nc.scalar.copy(scratch2[0:1, 1:2], dmy_ps[0:1, 48:49])
                # scores phase ------------------------------------------------
                scores_sb = mp.tile([H, LK], bf16, tag="scores", bufs=1)
                m8 = mp.tile([H, 2 * NT], f32, tag="m8", bufs=2)
                for t in range(NT):
                    kt = mp.tile([128, NCH, 1024], bf16, tag="kt", bufs=4)
                    nc.scalar.dma_start(
                        out=kt,
                        in_=kT_d[bl, :, t * 1024:(t + 1) * 1024]
                        .rearrange("(ch p) s -> p ch s", p=128))
                    ldw_k = nc.tensor.ldweights(kt[:, 0, 0:1])
                    prev_mm = None
                    for sh in range(2):
                        blk = t * 2 + sh
                        psum_s = pp.tile([33, 512], f32, tag="s")
                        dmy = nc.tensor.matmul(
                            dmy_ps[0:1, 0:1], kt[:, 0, 0:1], kt[:, 0, 0:1],
                            start=True, stop=True)
                        add_dep_helper(dmy.ins, (ldw_k if sh == 0 else prev_mm).ins,
                                       reason="absorb-chain")
                        for cj in range(NCH):
                            mm = nc.tensor.matmul(
                                psum_s[0:H, :],
                                rT_sb[:, cj, bl * H:(bl + 1) * H],
                                kt[:, cj, sh * 512:(sh + 1) * 512],
                                start=(cj == 0), stop=(cj == NCH - 1))
                            if cj == 0:
                                add_dep_helper(mm.ins, dmy.ins, reason="absorb")
                            prev_mm = mm
                        nc.vector.reduce_max(m8[:, blk:blk + 1], psum_s[0:H, :],
                                             axis=AX.X)
                        nc.vector.tensor_copy(
                            scores_sb[:, blk * 512:(blk + 1) * 512], psum_s[0:H, :])

                # scores-complete marker + ACT ratchet: placed a whole phase
                # upstream of the next batch's kt triggers so the scheduler's
                # DMA hoisting cannot lift them above it.
                nc.tensor.matmul(dmy_ps[0:1, 48:49], kt[0:1, 0, 0:1],
                                 kt[0:1, 0, 0:1], start=True, stop=True)
                nc.scalar.copy(scratch2[0:1, 4 + bl:5 + bl],
                               dmy_ps[0:1, 48:49])

                # softmax -----------------------------------------------------
                negmax = mp.tile([H, 1], f32, tag="negmax", bufs=2)
                nc.vector.reduce_max(negmax, m8, axis=AX.X, negate=True)
                attn_sb = mp.tile([H, LK], bf16, tag="attn", bufs=2)
                Zs = mp.tile([H, 2 * NT], f32, tag="Zs", bufs=2)
                for jj in range(2 * NT):
                    nc.scalar.activation(
                        attn_sb[:, jj * 512:(jj + 1) * 512],
                        scores_sb[:, jj * 512:(jj + 1) * 512],
                        FT.Exp, bias=negmax, scale=1.0,
                        accum_out=Zs[:, jj:jj + 1])
                Z = mp.tile([H, 1], f32, tag="Z", bufs=2)
                nc.vector.reduce_sum(Z, Zs, axis=AX.X)
                nc.vector.reciprocal(Zi2[bl * 32:bl * 32 + H, :], Z)

                # attn transposed on PE -> attnT [s, h]
                ldw_at = nc.tensor.ldweights(attn_sb[0:H, 0:1])
                dmy_at = nc.tensor.matmul(dmy_ps[0:1, 0:H], attn_sb[0:H, 0:1],
                                          idb[0:H, 0:H], start=True, stop=True)
                add_dep_helper(dmy_at.ins, ldw_at.ins, reason="absorb-chain")
                attnT_sb = mp.tile([128, LK // 128, H], bf16, tag="attnT", bufs=1)
                for g in range(2):
                    psum_at = pp.tile([128, 16, H], bf16, tag="tp")
                    for tt in range(16):
                        j = g * 16 + tt
                        tp_i = nc.tensor.transpose(
                            psum_at[:, tt, :],
                            attn_sb[0:H, j * 128:(j + 1) * 128], idb[0:H, 0:H])
                        add_dep_helper(tp_i.ins, dmy_at.ins, reason="absorb")
                    nc.vector.tensor_copy(
                        attnT_sb[:, g * 16:(g + 1) * 16, :], psum_at)
                    last_psum_at[0] = psum_at
                attnT_sb_prev[0] = attnT_sb

                # weighted-V phase -------------------------------------------
                pw = pp.tile([32, HID], f32, tag="w")
                psum_w[bl] = pw
                for t in range(NT):
                    vt = mp.tile([128, NCH, HID], bf16, tag="vt", bufs=4)
                    nc.scalar.dma_start(
                        out=vt,
                        in_=v_d[bl, t * 1024:(t + 1) * 1024, :]
                        .rearrange("(blk p) c -> p blk c", p=128))
                    ldw_v = nc.tensor.ldweights(vt[:, 0, 0:1])
                    dmy_v = nc.tensor.matmul(
                        dmy_ps[0:1, 0:1], vt[:, 0, 0:1], vt[:, 0, 0:1],
                        start=True, stop=True)
                    add_dep_helper(dmy_v.ins, ldw_v.ins, reason="absorb-chain")
                    for blk in range(NCH):
                        at_sl = attnT_sb[:, t * NCH + blk, :]
                        for hf in range(2):
                            mm = nc.tensor.matmul(
                                pw[0:H, hf * 512:(hf + 1) * 512],
                                at_sl,
                                vt[:, blk, hf * 512:(hf + 1) * 512],
                                start=(t == 0 and blk == 0),
                                stop=(t == NT - 1 and blk == NCH - 1))
                            if blk == 0 and hf == 0:
                                add_dep_helper(mm.ins, dmy_v.ins, reason="absorb")
                # V-phase-complete marker for the next batch's ACT ratchet
                nc.tensor.matmul(dmy_ps[0:1, 48:49], vt[0:1, 0, 0:1],
                                 vt[0:1, 0, 0:1], start=True, stop=True)

            # ---- batched tail: w -> u -> out for both batches at once ----
            w2_sb = mp.tile([HS, HID], bf16, tag="w2")
            nc.vector.memset(w2_sb, 0.0)
            for bl in range(BL):
                nc.vector.tensor_copy(w2_sb[bl * 32:bl * 32 + H, :],
                                      psum_w[bl][0:H, :])
            # ratchet ACT past batch 1's V accumulation, then pull the tail
            # weights into recycled stream-tile slots.
            nc.scalar.copy(scratch2[0:1, 3:4], dmy_ps[0:1, 48:49])
            Wv_sb = mp.tile([128, NCH, HID], bf16, tag="kt", bufs=4)
            nc.scalar.dma_start(
                out=Wv_sb, in_=Wv_d[:, :].rearrange("(ch p) h -> p ch h", p=128))
            Wo_sb = mp.tile([128, NCH, HID], bf16, tag="vt", bufs=4)
            nc.scalar.dma_start(
                out=Wo_sb, in_=Wo_d[:, :].rearrange("(ch p) h -> p ch h", p=128))
            ldw_w = nc.tensor.ldweights(w2_sb[0:HS, 0:1])
            dmy_w = nc.tensor.matmul(dmy_ps[0:1, 0:HS], w2_sb[0:HS, 0:1],
                                     idb[0:HS, 0:HS], start=True, stop=True)
            add_dep_helper(dmy_w.ins, ldw_w.ins, reason="absorb-chain")
            psum_wt = pp.tile([128, NCH, HS], bf16, tag="tp")
            for cj in range(NCH):
                tp_i = nc.tensor.transpose(
                    psum_wt[:, cj, :],
                    w2_sb[0:HS, cj * 128:(cj + 1) * 128], idb[0:HS, 0:HS])
                add_dep_helper(tp_i.ins, dmy_w.ins, reason="absorb")
            wT_sb = mp.tile([128, NCH, HS], bf16, tag="wT")
            nc.vector.tensor_copy(wT_sb, psum_wt)

            psum_u = pp.tile([HS, HID], f32, tag="w")
            ldw_wv = nc.tensor.ldweights(Wv_sb[:, 0, 0:1])
            dmy_u = nc.tensor.matmul(dmy_ps[0:1, 0:1], wT_sb[:, 0, 0:1],
                                     wT_sb[:, 0, 0:1], start=True, stop=True)
            add_dep_helper(dmy_u.ins, ldw_wv.ins, reason="absorb-chain")
            first_u = [True]
            for cj in range(NCH):
                for hf in range(2):
                    mm = nc.tensor.matmul(
                        psum_u[0:HS, hf * 512:(hf + 1) * 512],
                        wT_sb[:, cj, :],
                        Wv_sb[:, cj, hf * 512:(hf + 1) * 512],
                        start=(cj == 0), stop=(cj == NCH - 1))
                    if first_u[0]:
                        add_dep_helper(mm.ins, dmy_u.ins, reason="absorb")
                        first_u[0] = False
            um2_sb = mp.tile([HS, HID], bf16, tag="um2")
            nc.vector.tensor_copy(scratch[0:1, 3:4], psum_u[0:1, 0:1])
            nc.vector.scalar_tensor_tensor(
                um2_sb, psum_u[0:HS, :], Zi2, mask2,
                op0=mybir.AluOpType.mult, op1=mybir.AluOpType.mult)

            ldw_um = nc.tensor.ldweights(um2_sb[0:HS, 0:1])
            dmy_um = nc.tensor.matmul(dmy_ps[0:1, 0:HS], um2_sb[0:HS, 0:1],
                                      idb[0:HS, 0:HS], start=True, stop=True)
            add_dep_helper(dmy_um.ins, ldw_um.ins, reason="absorb-chain")
            psum_umt = pp.tile([128, NCH, 4, H], bf16, tag="tp")
            for m in range(NCH):
                tp_i = nc.tensor.transpose(
                    psum_umt[:, m, :, :],
                    um2_sb[0:HS, m * 128:(m + 1) * 128], idb[0:HS, 0:HS])
                add_dep_helper(tp_i.ins, dmy_um.ins, reason="absorb")
            umT_sb = mp.tile([128, NCH, 4, H], bf16, tag="umT")
            nc.vector.tensor_copy(umT_sb, psum_umt)
            uT_f = mp.tile([128, NCH, 4], f32, tag="uTf")
            nc.vector.reduce_sum(uT_f, umT_sb, axis=AX.X)
            uT_sb = mp.tile([128, NCH, 4], bf16, tag="uT")
            nc.vector.memset(uT_sb, 0.0)
            for b in range(BL):
                nc.vector.tensor_add(uT_sb[:, :, 2 * b], uT_f[:, :, 2 * b], bvT)

            psum_o = pp.tile([HS, HID], f32, tag="w")
            ldw_ut = nc.tensor.ldweights(uT_sb[:, 0, 0:1])
            ldw_wo = nc.tensor.ldweights(Wo_sb[:, 0, 0:1])
            add_dep_helper(ldw_wo.ins, ldw_ut.ins, reason="absorb-chain")
            first_o = [True]
            for m in range(NCH):
                for hf in range(2):
                    mm = nc.tensor.matmul(
                        psum_o[0:4, hf * 512:(hf + 1) * 512],
                        uT_sb[:, m, :],
                        Wo_sb[:, m, hf * 512:(hf + 1) * 512],
                        start=(m == 0), stop=False)
                    if first_o[0]:
                        add_dep_helper(mm.ins, ldw_wo.ins, reason="absorb")
                        first_o[0] = False
            for hf in range(2):
                nc.tensor.matmul(
                    psum_o[0:4, hf * 512:(hf + 1) * 512],
                    one14, boB[0:1, hf * 512:(hf + 1) * 512],
                    start=False, stop=(hf == 1))
            out_sb = mp.tile([4, HID], f32, tag="osb")
            nc.scalar.copy(out_sb, psum_o[0:4, :])
            prev_pad = None
            for x in range(4):
                p_i = nc.scalar.copy(scratch2[0:1, 6:7], out_sb[0:1, x:x + 1])
                if prev_pad is not None:
                    add_dep_helper(p_i.ins, prev_pad.ins, reason="act-pad-chain")
                prev_pad = p_i
            for b in range(BL):
                nc.scalar.dma_start(out=out_d[b:b + 1, :],
                                    in_=out_sb[2 * b:2 * b + 1, :])

    return nc


def make_in_maps(q, k, v, Wq, bq, Wv, bv, Wo, bo, Wk):
    scale = DH ** -0.5
    mask = np.zeros((H, HID), dtype=np.float32)
    for h in range(H):
        mask[h, h * DH:(h + 1) * DH] = 1.0
    zrow = np.zeros((32 - H, HID), dtype=np.float32)
    mask2 = np.concatenate([mask, zrow, mask, zrow], axis=0)
    blob = np.zeros((128, 3230), dtype=np.float32)
    blob[:, 0:128] = np.eye(128, dtype=np.float32)
    blob[0:HS, 128:128 + HID] = mask2
    blob[0:1, 1152:1152 + HID] = bo.reshape(1, HID)
    blob[0:1, 2176:2176 + HID] = (bq * scale).reshape(1, HID)
    blob[0:1, 3200:3200 + BL] = 1.0
    blob[0:1, 3202:3206] = np.array([1.0, 0.0, 1.0, 0.0])
    blob[:, 3206:3206 + NCH] = bv.reshape(NCH, 128).T
    shared = {
        "Wq": np.ascontiguousarray(Wq).astype(bf16_np),
        "WkT": np.ascontiguousarray(Wk.T).astype(bf16_np),
        "Wv": np.ascontiguousarray(Wv).astype(bf16_np),
        "Wo": np.ascontiguousarray(Wo).astype(bf16_np),
    }
    in_maps = []
    for c in range(NCORES):
        sl = slice(BL * c, BL * (c + 1))
        bc = blob.copy()
        qT = (q[sl] * scale).T  # [HID, BL]
        bc[:, 3214:3214 + NCH * BL] = qT.reshape(NCH, 128, BL).transpose(
            1, 0, 2).reshape(128, NCH * BL)
        in_maps.append({
            "kT_loc": np.ascontiguousarray(k[sl].transpose(0, 2, 1)).astype(bf16_np),
            "v_loc": np.ascontiguousarray(v[sl]).astype(bf16_np),
            "blob": bc.astype(bf16_np),
            **shared,
        })
    return in_maps


_nc_cache = None


def kernel(q, k, v, index_sample, Wq, bq, Wk, bk, Wv, bv, Wo, bo):
    global _nc_cache
    q, k, v = np.asarray(q, np.float32), np.asarray(k, np.float32), np.asarray(v, np.float32)
    Wq, bq = np.asarray(Wq, np.float32), np.asarray(bq, np.float32)
    Wk = np.asarray(Wk, np.float32)
    Wv, bv = np.asarray(Wv, np.float32), np.asarray(bv, np.float32)
    Wo, bo = np.asarray(Wo, np.float32), np.asarray(bo, np.float32)
    # bk provably cancels in the softmax (constant shift per (b, h) row).

    if _nc_cache is None:
        _nc_cache = build_nc()
    nc = _nc_cache
    in_maps = make_in_maps(q, k, v, Wq, bq, Wv, bv, Wo, bo, Wk)
    res = bass_utils.run_bass_kernel_spmd(nc, in_maps, core_ids=list(range(NCORES)))
    out = np.concatenate([r["out_loc"] for r in res.results], axis=0)
    return np.ascontiguousarray(out.astype(np.float32))
